# revision 2
# baseline (speedup 1.0000x reference)
"""Sliding-window causal self-attention, formulation B:
scores computed TRANSPOSED (ST[k,q]) so no P^T transpose matmuls are needed;
softmax row sums come free from a ones-column appended to V; normalization is
a partition-broadcast reciprocal multiply fused with the PSUM->SBUF drain.
Two passes over head pairs to fit the 8 PSUM banks; out-proj trails pass B.
"""

import math

import numpy as np

B = 2
T = 2048
C = 1024
H = 16
DH = 64
WINDOW = 256
HEADS_PER_CORE = 4
N_CORES = 8
QT = T // 128  # 16 tiles of 128 tokens
FQ = HEADS_PER_CORE * DH  # 256 local features

_PROGRAM = None
DEBUG = False


def _emit(nc, tc, aps, ctx):
    from concourse import mybir

    f32 = mybir.dt.float32
    bf16 = mybir.dt.bfloat16
    Exp = mybir.ActivationFunctionType.Exp

    xT, wT, woT, cos4, sin4, bmask, y = (
        aps["xT"], aps["wT"], aps["woT"], aps["cos4"], aps["sin4"],
        aps["bmask"], aps["y"],
    )
    dbg = {k: v for k, v in aps.items() if k.startswith("dbg_")}

    consts = ctx.enter_context(tc.tile_pool(name="consts", bufs=1))
    stage = ctx.enter_context(tc.tile_pool(name="stage", bufs=1))
    tmp = ctx.enter_context(tc.tile_pool(name="tmp", bufs=2))
    rcpp = ctx.enter_context(tc.tile_pool(name="rcpp", bufs=4))
    ysbp = ctx.enter_context(tc.tile_pool(name="ysbp", bufs=2))

    # ---- resident inputs ----
    wT_sb = consts.tile([128, 8 * 768], bf16, tag="wT")
    wv_ = wT_sb.rearrange("p (kc f) -> p kc f", kc=8)
    ws_ = wT.rearrange("(kc p) f -> p kc f", p=128)
    nc.sync.dma_start(out=wv_[:, 0:4], in_=ws_[:, 0:4])
    xT_sb = consts.tile([128, 8 * T], bf16, tag="xT")  # [C-part, (kc t)]
    xv = xT_sb.rearrange("p (kc t) -> p kc t", kc=8)
    xs = xT.rearrange("(kc p) t -> p kc t", p=128)
    nc.sync.dma_start(out=xv[:, :, 0:512], in_=xs[:, :, 0:512])
    cos_sb = consts.tile([128, T], bf16, tag="cos")
    nc.scalar.dma_start(out=cos_sb, in_=cos4)
    sin_sb = consts.tile([128, T], bf16, tag="sin")
    nc.scalar.dma_start(out=sin_sb, in_=sin4)
    nc.sync.dma_start(out=wv_[:, 4:8], in_=ws_[:, 4:8])
    for s in range(1, 4):
        tsl = slice(s * 512, (s + 1) * 512)
        nc.sync.dma_start(out=xv[:, :, tsl], in_=xs[:, :, tsl])
    bmask_sb = consts.tile([128, 256], bf16, tag="bmask")
    nc.scalar.dma_start(out=bmask_sb, in_=bmask)
    woT_sb = consts.tile([128, 2 * C], bf16, tag="woT")
    nc.scalar.dma_start(
        out=woT_sb.rearrange("p (kc e) -> p kc e", kc=2),
        in_=woT.rearrange("(kc p) e -> p kc e", p=128),
    )

    # ---- persistent intermediates ----
    pre = [stage.tile([128, T], bf16, tag=f"pre{i}", name=f"pre{i}")
           for i in range(4)]
    rot = [stage.tile([128, T], bf16, tag=f"rot{i}", name=f"rot{i}")
           for i in range(4)]
    qhT = stage.tile([64, HEADS_PER_CORE * T], bf16, tag="qhT")
    khT = stage.tile([64, HEADS_PER_CORE * T], bf16, tag="khT")
    # v with 64 ones columns per (key-tile, head): the o-matmul then
    # emits rowsums broadcast across PSUM rows 64:128 for free
    v_sb = stage.tile([128, QT * HEADS_PER_CORE * 128], bf16, tag="v")
    vv = v_sb.rearrange("p (k h c) -> p k h c", h=HEADS_PER_CORE, c=128)
    # exp(ST) slabs, one per (kt, head-in-pair): [128, (kt j 384)]
    slab = stage.tile([128, QT * 2 * 384], bf16, tag="slab")
    attnP = [stage.tile([128, T], bf16, tag=f"attn{p}", name=f"attn{p}")
             for p in range(2)]

    nc.gpsimd.memset(vv[:, :, :, 64:128], 1.0)

    # ---- phase 1: q/k projection + RoPE + repack ----
    with tc.tile_pool(name="pmm", bufs=2, space="PSUM") as pmm:
        for s in range(4):
            tsl = slice(s * 512, (s + 1) * 512)
            for blk in range(4):  # q_x1 q_x2 k_x1 k_x2
                acc = pmm.tile([128, 512], f32, tag=f"mm{blk}")
                for kc in range(8):
                    nc.tensor.matmul(
                        acc,
                        lhsT=wT_sb[:, kc * 768 + blk * 128:kc * 768 + (blk + 1) * 128],
                        rhs=xT_sb[:, kc * T + s * 512:kc * T + (s + 1) * 512],
                        start=(kc == 0),
                        stop=(kc == 7),
                    )
                nc.any.tensor_copy(pre[blk][:, tsl], acc)
            for pair in range(2):  # 0 -> q, 1 -> k
                x1, x2 = pre[2 * pair][:, tsl], pre[2 * pair + 1][:, tsl]
                r1, r2 = rot[2 * pair][:, tsl], rot[2 * pair + 1][:, tsl]
                t1 = tmp.tile([128, 512], bf16, tag="t1")
                t2 = tmp.tile([128, 512], bf16, tag="t2")
                t3 = tmp.tile([128, 512], bf16, tag="t3")
                t4 = tmp.tile([128, 512], bf16, tag="t4")
                nc.vector.tensor_mul(t1, x1, cos_sb[:, tsl])
                nc.vector.tensor_mul(t2, x2, sin_sb[:, tsl])
                nc.vector.tensor_sub(r1, t1, t2)
                nc.gpsimd.tensor_mul(t3, x2, cos_sb[:, tsl])
                nc.gpsimd.tensor_mul(t4, x1, sin_sb[:, tsl])
                nc.vector.tensor_add(r2, t3, t4)
        # repack rotated q/k into head-major [64, (h t)] layout; SBUF APs
        # cannot cross partitions in a free dim, so one DMA per (head, half)
        for hl in range(HEADS_PER_CORE):
            for half in range(2):
                eng = nc.sync if hl % 2 == 0 else nc.scalar
                eng.dma_start(
                    out=qhT[half * 32:(half + 1) * 32, hl * T:(hl + 1) * T],
                    in_=rot[half][hl * 32:(hl + 1) * 32, :],
                )
                eng.dma_start(
                    out=khT[half * 32:(half + 1) * 32, hl * T:(hl + 1) * T],
                    in_=rot[2 + half][hl * 32:(hl + 1) * 32, :],
                )

    # ---- phase 2: v tiles in token-major layout ----
    with tc.tile_pool(name="pv", bufs=2, space="PSUM") as pv:
        for kt in range(QT):
            vacc = pv.tile([128, FQ], f32, tag="v")
            for kc in range(8):
                nc.tensor.matmul(
                    vacc,
                    lhsT=xT_sb[:, kc * T + kt * 128:kc * T + (kt + 1) * 128],
                    rhs=wT_sb[:, kc * 768 + 512:kc * 768 + 768],
                    start=(kc == 0),
                    stop=(kc == 7),
                )
            nc.any.tensor_copy(
                vv[:, kt, :, 0:64],
                vacc.rearrange("p (h c) -> p h c", h=HEADS_PER_CORE),
            )

    # ---- phase 3: attention (two passes over head pairs) + out-proj ----
    slabv = slab.rearrange("p (k j c) -> p k j c", k=QT, j=2)
    bmv = bmask_sb.rearrange("p (b c) -> p b c", b=2)

    with tc.tile_pool(name="pst", bufs=2, space="PSUM") as pst, \
         tc.tile_pool(name="po", bufs=2, space="PSUM") as po, \
         tc.tile_pool(name="py", bufs=2, space="PSUM") as py:
        for p in range(2):  # head pair (kc half): heads 2p, 2p+1
            for kt in range(QT):
                qhi = min(kt + 2, QT - 1)
                w = (qhi - kt + 1) * 128
                stp = pst.tile([128, 1024], f32, tag="st")
                for j in range(2):
                    hl = 2 * p + j
                    nc.tensor.matmul(
                        stp[:, j * 512:j * 512 + w],
                        lhsT=khT[:, hl * T + kt * 128:hl * T + (kt + 1) * 128],
                        rhs=qhT[:, hl * T + kt * 128:hl * T + kt * 128 + w],
                        start=True,
                        stop=True,
                    )
                sl = slabv[:, kt]  # [128, 2, 384]
                nc.scalar.activation(
                    sl[:, :, 0:w],
                    stp.rearrange("p (j c) -> p j c", j=2)[:, :, 0:w],
                    Exp,
                )
                # binary mask post-exp: diag block always, upper-2 if present
                for j in range(2):
                    meng = nc.gpsimd if (2 * kt + j) % 4 == 3 else nc.vector
                    if qhi == kt + 2:
                        sv = sl[:, j].rearrange("p (b c) -> p b c", b=3)[:, 0::2, :]
                        meng.tensor_mul(sv, sv, bmv)
                    else:
                        meng.tensor_mul(
                            sl[:, j, 0:128], sl[:, j, 0:128], bmv[:, 0])
                # o for qt = kt: one PSUM bank, sequential j groups of
                # back-to-back accumulating matmuls (whole-bank has_written
                # clearing on start makes interleaved groups illegal)
                qt = kt
                klo = max(0, qt - 2)
                o_t = po.tile([128, 256], f32, tag="o")
                for j in range(2):
                    hl = 2 * p + j
                    for kt2 in range(klo, qt + 1):
                        dq = qt - kt2
                        nc.tensor.matmul(
                            o_t[:, j * 128:(j + 1) * 128],
                            lhsT=v_sb[:, (kt2 * 4 + hl) * 128:(kt2 * 4 + hl + 1) * 128],
                            rhs=slabv[:, kt2, j, dq * 128:(dq + 1) * 128],
                            start=(kt2 == klo),
                            stop=(kt2 == qt),
                        )
                rs_sb = rcpp.tile([64, 256], f32, tag="rs")
                nc.scalar.copy(rs_sb, o_t[64:128, :])
                rcp = rcpp.tile([64, 256], f32, tag="rcp")
                nc.vector.reciprocal_approx_fast(rcp, rs_sb)
                if dbg:
                    od = rcpp.tile([128, 256], bf16, tag="od", name="od")
                    nc.vector.tensor_copy(od, o_t)
                    nc.sync.dma_start(
                        out=dbg["dbg_o"][:, (p * QT + qt) * 256:(p * QT + qt + 1) * 256],
                        in_=od)
                    rd = rcpp.tile([4, 256], f32, tag="rd", name="rd")
                    nc.vector.tensor_copy(rd, rcp[0:4])
                    nc.sync.dma_start(
                        out=dbg["dbg_rcp"][:, (p * QT + qt) * 256:(p * QT + qt + 1) * 256],
                        in_=rd)
                for j in range(2):
                    nc.vector.tensor_mul(
                        attnP[p][j * 64:(j + 1) * 64, qt * 128:(qt + 1) * 128],
                        o_t[0:64, j * 128:(j + 1) * 128],
                        rcp[:, j * 128:(j + 1) * 128],
                    )
                if p == 1:
                    _emit_outproj(nc, py, attnP, woT_sb, ysbp, y, qt, f32, bf16)
    if dbg:
        nc.sync.dma_start(out=dbg["dbg_qhT"], in_=qhT)
        nc.sync.dma_start(out=dbg["dbg_khT"], in_=khT)
        nc.sync.dma_start(out=dbg["dbg_v"], in_=v_sb)
        nc.sync.dma_start(out=dbg["dbg_slab"], in_=slab)
        nc.sync.dma_start(out=dbg["dbg_attn0"], in_=attnP[0])
        nc.sync.dma_start(out=dbg["dbg_attn1"], in_=attnP[1])


def _emit_outproj(nc, py, attnP, woT_sb, ysbp, y, qt, f32, bf16):
    ysb = ysbp.tile([128, C], bf16, tag="ysb")
    for nh in range(2):
        acc = py.tile([128, 512], f32, tag="y")
        for kc in range(2):
            nc.tensor.matmul(
                acc,
                lhsT=attnP[kc][:, qt * 128:(qt + 1) * 128],
                rhs=woT_sb[:, kc * C + nh * 512:kc * C + (nh + 1) * 512],
                start=(kc == 0),
                stop=(kc == 1),
            )
        if (qt + nh) % 2 == 0:
            nc.scalar.copy(ysb[:, nh * 512:(nh + 1) * 512], acc)
        else:
            nc.vector.tensor_copy(ysb[:, nh * 512:(nh + 1) * 512], acc)
    nc.sync.dma_start(out=y[qt * 128:(qt + 1) * 128, :], in_=ysb)


def _build_program():
    import concourse.tile as tile
    from concourse import bacc, mybir

    bf16 = mybir.dt.bfloat16

    nc = bacc.Bacc("TRN2", target_bir_lowering=False, debug=False,
                   num_devices=N_CORES)
    aps = {
        "xT": nc.dram_tensor("xT", [C, T], bf16, kind="ExternalInput").ap(),
        "wT": nc.dram_tensor("wT", [C, 768], bf16, kind="ExternalInput").ap(),
        "woT": nc.dram_tensor("woT", [FQ, C], bf16, kind="ExternalInput").ap(),
        "cos4": nc.dram_tensor("cos4", [128, T], bf16, kind="ExternalInput").ap(),
        "sin4": nc.dram_tensor("sin4", [128, T], bf16, kind="ExternalInput").ap(),
        "bmask": nc.dram_tensor("bmask", [128, 256], bf16, kind="ExternalInput").ap(),
        "y": nc.dram_tensor("y", [T, C], bf16, kind="ExternalOutput").ap(),
    }
    if DEBUG:
        for nm, shp in [("dbg_qhT", [64, 4 * T]), ("dbg_khT", [64, 4 * T]),
                        ("dbg_v", [128, QT * 4 * 128]),
                        ("dbg_slab", [128, QT * 2 * 384]),
                        ("dbg_attn0", [128, T]), ("dbg_attn1", [128, T])]:
            aps[nm] = nc.dram_tensor(nm, shp, bf16, kind="ExternalOutput").ap()
        aps["dbg_o"] = nc.dram_tensor(
            "dbg_o", [128, 2 * QT * 256], bf16, kind="ExternalOutput").ap()
        aps["dbg_rcp"] = nc.dram_tensor(
            "dbg_rcp", [4, 2 * QT * 256], mybir.dt.float32,
            kind="ExternalOutput").ap()
    from contextlib import ExitStack

    with tile.TileContext(nc) as tc, ExitStack() as ctx:
        _emit(nc, tc, aps, ctx)
    nc.compile()
    return nc


def _get_program():
    global _PROGRAM
    if _PROGRAM is None:
        _PROGRAM = _build_program()
    return _PROGRAM


def _host_inputs(x, w_qkv, w_out):
    import ml_dtypes

    bf16 = ml_dtypes.bfloat16
    x = np.asarray(x, np.float32)
    w_qkv = np.asarray(w_qkv, np.float32)
    w_out = np.asarray(w_out, np.float32)

    wq, wk, wv = w_qkv[0:C], w_qkv[C:2 * C], w_qkv[2 * C:3 * C]
    scale = 1.0 / math.sqrt(DH)

    inv_freq = 1.0 / (10000.0 ** (np.arange(0, DH, 2, dtype=np.float32) / DH))
    freqs = np.outer(np.arange(T, dtype=np.float32), inv_freq)  # [T, 32]
    cos4 = np.ascontiguousarray(np.tile(np.cos(freqs).T, (4, 1))).astype(bf16)
    sin4 = np.ascontiguousarray(np.tile(np.sin(freqs).T, (4, 1))).astype(bf16)

    # binary masks in ST[k, q] layout: [diag (q >= k) | upper2 (q < k)]
    k = np.arange(128)[:, None]
    q = np.arange(128)[None, :]
    m_diag = (q >= k).astype(np.float32)
    m_up2 = (q < k).astype(np.float32)
    bmask = np.ascontiguousarray(
        np.concatenate([m_diag, m_up2], axis=1)).astype(bf16)

    xT = [np.ascontiguousarray(x[b].T).astype(bf16) for b in range(B)]

    in_maps = []
    for core in range(N_CORES):
        b, g = divmod(core, 4)
        hs = range(4 * g, 4 * g + 4)
        rows = []
        for half in range(2):  # q_x1, q_x2
            rows.append(np.concatenate(
                [wq[h * DH + 32 * half:h * DH + 32 * half + 32] for h in hs]) * scale)
        for half in range(2):  # k_x1, k_x2
            rows.append(np.concatenate(
                [wk[h * DH + 32 * half:h * DH + 32 * half + 32] for h in hs]))
        rows.append(wv[g * FQ:(g + 1) * FQ])
        wmat = np.concatenate(rows)  # [768, C]
        wTm = np.ascontiguousarray(wmat.T).astype(bf16)
        woTm = np.ascontiguousarray(w_out[:, g * FQ:(g + 1) * FQ].T).astype(bf16)
        in_maps.append({
            "xT": xT[b], "wT": wTm, "woT": woTm,
            "cos4": cos4, "sin4": sin4, "bmask": bmask,
        })
    return in_maps


def kernel(x, w_qkv, w_out, _trace=False):
    from concourse import bass_utils

    nc = _get_program()
    in_maps = _host_inputs(x, w_qkv, w_out)
    res = bass_utils.run_bass_kernel_spmd(
        nc, in_maps, core_ids=list(range(N_CORES)), trace=_trace,
    )
    parts = [res.results[core]["y"].astype(np.float32) for core in range(N_CORES)]
    out = np.stack([
        parts[0] + parts[1] + parts[2] + parts[3],
        parts[4] + parts[5] + parts[6] + parts[7],
    ])
    if _trace:
        return out, res
    return out
